# revision 4
# baseline (speedup 1.0000x reference)
"""Bass/Trainium2 kernel for nn_Channel_attention (bottom-16 channel gather).

reference semantics (per sample b):
    weight = mean(x[b], axis=(H, W))           # [C]
    idx    = argsort(weight)[:16]              # ascending pooled value
    out[b] = x[b, idx]                         # [16, H, W]

Strategy: pure data parallel, B=16 sharded 2 samples per core over 8 cores.
Per core (x shard viewed as [512, 16384] = [(sample, channel), H*W]):
  1. Stream [128ch, 4096] tiles, DVE reduce_add -> per-channel partial sums.
  2. Per sample (pipelined so sample 0's tail hides under sample 1's loads):
     PE-transpose its sums into a [1, 256] row; negate; two rounds of
     max8/max_index/match_replace -> bottom-16 channel indices in ascending
     order of pooled sum (argsort of sum == argsort of mean).
  3. Expand the 16 indices to 128 row-indices (idx*8 + subrow) with two tiny
     PE matmuls, then SWDGE indirect-gather [128, 2048] and store
     contiguously to the output.
"""

import sys

if "/opt/trn_rl_repo" not in sys.path:
    sys.path.insert(0, "/opt/trn_rl_repo")

import numpy as np

from concourse import bacc, mybir, tile
from concourse.bass import IndirectOffsetOnAxis
from concourse.bass_utils import run_bass_kernel_spmd
from concourse.masks import make_identity

N_CORES = 8
B, C, H, W = 16, 256, 128, 128
K = 16
BPC = B // N_CORES          # samples per core = 2
E = H * W                   # 16384 elems per channel
CH = 4096                   # load-chunk width (2 MiB tiles)
NJ = E // CH                # 4 chunks per (sample, channel-half)
GR = E // 2048              # gather sub-rows per channel (8 x 8KiB)
ROWS = BPC * C              # 512 channel rows per core

f32 = mybir.dt.float32
i32 = mybir.dt.int32
u32 = mybir.dt.uint32
X = mybir.AxisListType.X
Alu = mybir.AluOpType

_cache = {}


def _build():
    nc = bacc.Bacc("TRN2", target_bir_lowering=False, debug=False,
                   num_devices=N_CORES)
    x_d = nc.dram_tensor("x", [ROWS, E], f32, kind="ExternalInput")
    y_d = nc.dram_tensor("y", [BPC * K * GR, 2048], f32, kind="ExternalOutput")

    with tile.TileContext(nc) as tc:
        with (
            tc.tile_pool(name="load", bufs=8) as load_pool,
            tc.tile_pool(name="small", bufs=1) as small,
            tc.tile_pool(name="gather", bufs=2) as gather_pool,
            tc.tile_pool(name="psum", bufs=1, space="PSUM") as psum,
        ):
            # ---- constants (no deps; scheduler fills gaps with these) ----
            ident = small.tile([128, 128], f32)
            make_identity(nc, ident[:])

            e_i = small.tile([K, 128], i32)
            nc.gpsimd.iota(out=e_i[:], pattern=[[1, 128]], base=0,
                           channel_multiplier=0)
            nc.vector.tensor_scalar(out=e_i[:], in0=e_i[:], scalar1=3,
                                    scalar2=None, op0=Alu.arith_shift_right)
            e_f = small.tile([K, 128], f32)
            nc.vector.tensor_copy(e_f[:], e_i[:])
            col_i = small.tile([K, 1], i32)
            nc.gpsimd.iota(out=col_i[:], pattern=[[1, 1]], base=0,
                           channel_multiplier=1)
            col_f = small.tile([K, 1], f32)
            nc.vector.tensor_copy(col_f[:], col_i[:])
            e_mat = small.tile([K, 128], f32)
            nc.vector.tensor_scalar(out=e_mat[:], in0=e_f[:], scalar1=col_f[:],
                                    scalar2=None, op0=Alu.is_equal)

            pp = small.tile([128, 1], i32)
            nc.gpsimd.iota(out=pp[:], pattern=[[1, 1]], base=0,
                           channel_multiplier=1)
            nc.vector.tensor_scalar(out=pp[:], in0=pp[:], scalar1=7,
                                    scalar2=None, op0=Alu.bitwise_and)
            a7f = small.tile([128, 1], f32)
            nc.vector.tensor_copy(a7f[:], pp[:])

            xg = x_d[:].rearrange("r (u e) -> (r u) e", u=GR)

            # ---- per-sample pipeline ----
            for s in range(BPC):
                partials = small.tile([128, 2 * NJ], f32, tag=f"partials{s}")
                for h in range(2):
                    base = s * C + h * 128
                    for j in range(NJ):
                        t = load_pool.tile([128, CH], f32)
                        nc.sync.dma_start(
                            out=t[:], in_=x_d[base:base + 128,
                                              j * CH:(j + 1) * CH])
                        nc.vector.reduce_sum(
                            out=partials[:, h * NJ + j:h * NJ + j + 1],
                            in_=t[:], axis=X)

                sums = small.tile([128, 2], f32, tag=f"sums{s}")
                for h in range(2):
                    nc.vector.reduce_sum(out=sums[:, h:h + 1],
                                         in_=partials[:, h * NJ:(h + 1) * NJ],
                                         axis=X)

                # sums -> one [1, 256] row, negated
                psum_w = psum.tile([1, C], f32, tag=f"psw{s}")
                nc.tensor.matmul(out=psum_w[:, 0:128], lhsT=sums[:, 0:1],
                                 rhs=ident[:], start=True, stop=True)
                nc.tensor.matmul(out=psum_w[:, 128:256], lhsT=sums[:, 1:2],
                                 rhs=ident[:], start=True, stop=True)
                w_neg = small.tile([1, C], f32, tag=f"wneg{s}")
                nc.scalar.mul(w_neg[:], psum_w[:], -1.0)

                # bottom-16 via two rounds of max8 on -sums
                m1 = small.tile([1, 8], f32, tag=f"m1_{s}")
                m2 = small.tile([1, 8], f32, tag=f"m2_{s}")
                idx_u = small.tile([1, K], u32, tag=f"idxu{s}")
                w_rep = small.tile([1, C], f32, tag=f"wrep{s}")
                nc.vector.max(out=m1[:], in_=w_neg[:])
                nc.vector.max_index(out=idx_u[:, 0:8], in_max=m1[:],
                                    in_values=w_neg[:])
                nc.vector.match_replace(out=w_rep[:], in_to_replace=m1[:],
                                        in_values=w_neg[:], imm_value=-1e38)
                nc.vector.max(out=m2[:], in_=w_rep[:])
                nc.vector.max_index(out=idx_u[:, 8:16], in_max=m2[:],
                                    in_values=w_rep[:])
                idx_f = small.tile([1, K], f32, tag=f"idxf{s}")
                nc.vector.tensor_copy(idx_f[:], idx_u[:])

                # expand to 128 gather-row indices:
                # idx128[p] = (s*C + idx[p>>3])*8 + (p&7)
                psum_t = psum.tile([K, 1], f32, tag=f"pst{s}")
                nc.tensor.matmul(out=psum_t[:], lhsT=idx_f[:],
                                 rhs=ident[0:1, 0:1], start=True, stop=True)
                idx_t = small.tile([K, 1], f32, tag=f"idxt{s}")
                nc.vector.tensor_copy(idx_t[:], psum_t[:])
                psum_e = psum.tile([128, 1], f32, tag=f"pse{s}")
                nc.tensor.matmul(out=psum_e[:], lhsT=e_mat[:], rhs=idx_t[:],
                                 start=True, stop=True)

                idx128_f = small.tile([128, 1], f32, tag=f"i128f{s}")
                nc.vector.tensor_scalar(out=idx128_f[:], in0=psum_e[:],
                                        scalar1=float(GR),
                                        scalar2=float(s * C * GR),
                                        op0=Alu.mult, op1=Alu.add)
                nc.vector.tensor_tensor(out=idx128_f[:], in0=idx128_f[:],
                                        in1=a7f[:], op=Alu.add)
                idx128_i = small.tile([128, 1], i32, tag=f"i128i{s}")
                nc.vector.tensor_copy(idx128_i[:], idx128_f[:])

                # gather the selected channels, store contiguously
                g = gather_pool.tile([128, 2048], f32, tag=f"g{s}")
                nc.gpsimd.indirect_dma_start(
                    out=g[:], out_offset=None, in_=xg,
                    in_offset=IndirectOffsetOnAxis(ap=idx128_i[:], axis=0))
                nc.sync.dma_start(out=y_d[s * 128:(s + 1) * 128, :], in_=g[:])

    nc.compile()
    return nc


def get_nc():
    if "nc" not in _cache:
        _cache["nc"] = _build()
    return _cache["nc"]


def make_in_maps(x: np.ndarray) -> list[dict[str, np.ndarray]]:
    x = np.ascontiguousarray(np.asarray(x, dtype=np.float32))
    assert x.shape == (B, C, H, W)
    return [{"x": x[c * BPC:(c + 1) * BPC].reshape(ROWS, E)}
            for c in range(N_CORES)]


def assemble(results: list[dict[str, np.ndarray]]) -> np.ndarray:
    out = np.empty((B, K, H, W), dtype=np.float32)
    for c in range(N_CORES):
        out[c * BPC:(c + 1) * BPC] = results[c]["y"].reshape(BPC, K, H, W)
    return out


def kernel(x: np.ndarray) -> np.ndarray:
    nc = get_nc()
    res = run_bass_kernel_spmd(nc, make_in_maps(x), list(range(N_CORES)))
    return assemble(res.results)


# revision 5
# speedup vs baseline: 1.0564x; 1.0564x over previous
"""Bass/Trainium2 kernel for nn_Channel_attention (bottom-16 channel gather).

reference semantics (per sample b):
    weight = mean(x[b], axis=(H, W))           # [C]
    idx    = argsort(weight)[:16]              # ascending pooled value
    out[b] = x[b, idx]                         # [16, H, W]

Strategy: pure data parallel, B=16 sharded 2 samples per core over 8 cores.
Per core (x shard viewed as [512, 16384] = [(sample, channel), H*W]):
  1. Stream [128ch, 2048] tiles, DVE reduce_add -> per-channel partial sums.
     Load DMAs alternate between the sync and scalar HWDGE queues.
  2. Per sample (pipelined so sample 0's tail hides under sample 1's loads):
     negate sums on DVE, PE-transpose into a [1, 256] row, two rounds of
     max8/max_index/match_replace -> bottom-16 channel indices in ascending
     order of pooled sum (argsort of sum == argsort of mean).
  3. Expand the 16 indices to 128 row-indices (idx*8 + subrow) with two tiny
     PE matmuls, then SWDGE indirect-gather [128, 2048] and store
     contiguously to the output (split across both HWDGE queues).
"""

import sys

if "/opt/trn_rl_repo" not in sys.path:
    sys.path.insert(0, "/opt/trn_rl_repo")

import numpy as np

from concourse import bacc, mybir, tile
from concourse.bass import IndirectOffsetOnAxis
from concourse.bass_utils import run_bass_kernel_spmd
from concourse.masks import make_identity

N_CORES = 8
B, C, H, W = 16, 256, 128, 128
K = 16
BPC = B // N_CORES          # samples per core = 2
E = H * W                   # 16384 elems per channel
GR = 8                      # gather sub-rows per channel (8 x 8KiB)
ROWS = BPC * C              # 512 channel rows per core

f32 = mybir.dt.float32
i32 = mybir.dt.int32
u32 = mybir.dt.uint32
X = mybir.AxisListType.X
Alu = mybir.AluOpType

# chunk widths per (sample, half); last half of the last sample ends with
# small chunks so the final reduce exits quickly after the last load lands
CHUNKS = [2048] * 8
CHUNKS_LAST = [2048] * 7 + [1024, 1024]

_cache = {}


def _build():
    nc = bacc.Bacc("TRN2", target_bir_lowering=False, debug=False,
                   num_devices=N_CORES)
    x_d = nc.dram_tensor("x", [ROWS, E], f32, kind="ExternalInput")
    y_d = nc.dram_tensor("y", [BPC * K * GR, 2048], f32, kind="ExternalOutput")

    with tile.TileContext(nc) as tc:
        with (
            tc.tile_pool(name="load", bufs=20) as load_pool,
            tc.tile_pool(name="small", bufs=1) as small,
            tc.tile_pool(name="gather", bufs=2) as gather_pool,
            tc.tile_pool(name="psum", bufs=1, space="PSUM") as psum,
        ):
            # ---- constants (no deps; scheduler fills gaps with these) ----
            ident = small.tile([128, 128], f32)
            make_identity(nc, ident[:])

            e_i = small.tile([K, 128], i32)
            nc.gpsimd.iota(out=e_i[:], pattern=[[1, 128]], base=0,
                           channel_multiplier=0)
            nc.vector.tensor_scalar(out=e_i[:], in0=e_i[:], scalar1=3,
                                    scalar2=None, op0=Alu.arith_shift_right)
            e_f = small.tile([K, 128], f32)
            nc.vector.tensor_copy(e_f[:], e_i[:])
            col_i = small.tile([K, 1], i32)
            nc.gpsimd.iota(out=col_i[:], pattern=[[1, 1]], base=0,
                           channel_multiplier=1)
            col_f = small.tile([K, 1], f32)
            nc.vector.tensor_copy(col_f[:], col_i[:])
            e_mat = small.tile([K, 128], f32)
            nc.vector.tensor_scalar(out=e_mat[:], in0=e_f[:], scalar1=col_f[:],
                                    scalar2=None, op0=Alu.is_equal)

            pp = small.tile([128, 1], i32)
            nc.gpsimd.iota(out=pp[:], pattern=[[1, 1]], base=0,
                           channel_multiplier=1)
            nc.vector.tensor_scalar(out=pp[:], in0=pp[:], scalar1=7,
                                    scalar2=None, op0=Alu.bitwise_and)
            a7f = small.tile([128, 1], f32)
            nc.vector.tensor_copy(a7f[:], pp[:])

            xg = x_d[:].rearrange("r (u e) -> (r u) e", u=GR)
            dma_engines = [nc.sync, nc.scalar]
            n_dma = 0

            # ---- per-sample pipeline ----
            for s in range(BPC):
                ncols = 0
                chunk_lists = []
                for h in range(2):
                    cl = CHUNKS_LAST if (s == BPC - 1 and h == 1) else CHUNKS
                    chunk_lists.append(cl)
                    ncols = max(ncols, len(cl))
                partials = small.tile([128, 2 * ncols], f32, tag=f"partials{s}")

                for h in range(2):
                    base = s * C + h * 128
                    off = 0
                    for j, cw in enumerate(chunk_lists[h]):
                        t = load_pool.tile([128, 2048], f32)
                        eng = dma_engines[n_dma % 2]
                        n_dma += 1
                        eng.dma_start(out=t[:, 0:cw],
                                      in_=x_d[base:base + 128, off:off + cw])
                        nc.vector.reduce_sum(
                            out=partials[:, h * ncols + j:h * ncols + j + 1],
                            in_=t[:, 0:cw], axis=X)
                        off += cw

                sums = small.tile([128, 2], f32, tag=f"sums{s}")
                for h in range(2):
                    nj = len(chunk_lists[h])
                    nc.vector.reduce_sum(
                        out=sums[:, h:h + 1],
                        in_=partials[:, h * ncols:h * ncols + nj],
                        axis=X, negate=True)

                # negated sums -> one [1, 256] row via PE transpose
                psum_w = psum.tile([1, C], f32, tag=f"psw{s}")
                nc.tensor.matmul(out=psum_w[:, 0:128], lhsT=sums[:, 0:1],
                                 rhs=ident[:], start=True, stop=True)
                nc.tensor.matmul(out=psum_w[:, 128:256], lhsT=sums[:, 1:2],
                                 rhs=ident[:], start=True, stop=True)
                w_neg = small.tile([1, C], f32, tag=f"wneg{s}")
                nc.vector.tensor_copy(w_neg[:], psum_w[:])

                # bottom-16 via two rounds of max8 on -sums
                m1 = small.tile([1, 8], f32, tag=f"m1_{s}")
                m2 = small.tile([1, 8], f32, tag=f"m2_{s}")
                idx_u = small.tile([1, K], u32, tag=f"idxu{s}")
                w_rep = small.tile([1, C], f32, tag=f"wrep{s}")
                nc.vector.max(out=m1[:], in_=w_neg[:])
                nc.vector.max_index(out=idx_u[:, 0:8], in_max=m1[:],
                                    in_values=w_neg[:])
                nc.vector.match_replace(out=w_rep[:], in_to_replace=m1[:],
                                        in_values=w_neg[:], imm_value=-1e38)
                nc.vector.max(out=m2[:], in_=w_rep[:])
                nc.vector.max_index(out=idx_u[:, 8:16], in_max=m2[:],
                                    in_values=w_rep[:])
                idx_f = small.tile([1, K], f32, tag=f"idxf{s}")
                nc.vector.tensor_copy(idx_f[:], idx_u[:])

                # expand to 128 gather-row indices:
                # idx128[p] = (s*C + idx[p>>3])*8 + (p&7)
                psum_t = psum.tile([K, 1], f32, tag=f"pst{s}")
                nc.tensor.matmul(out=psum_t[:], lhsT=idx_f[:],
                                 rhs=ident[0:1, 0:1], start=True, stop=True)
                idx_t = small.tile([K, 1], f32, tag=f"idxt{s}")
                nc.vector.tensor_copy(idx_t[:], psum_t[:])
                psum_e = psum.tile([128, 1], f32, tag=f"pse{s}")
                nc.tensor.matmul(out=psum_e[:], lhsT=e_mat[:], rhs=idx_t[:],
                                 start=True, stop=True)

                idx128_f = small.tile([128, 1], f32, tag=f"i128f{s}")
                nc.vector.tensor_scalar(out=idx128_f[:], in0=psum_e[:],
                                        scalar1=float(GR),
                                        scalar2=float(s * C * GR),
                                        op0=Alu.mult, op1=Alu.add)
                nc.vector.tensor_tensor(out=idx128_f[:], in0=idx128_f[:],
                                        in1=a7f[:], op=Alu.add)
                idx128_i = small.tile([128, 1], i32, tag=f"i128i{s}")
                nc.vector.tensor_copy(idx128_i[:], idx128_f[:])

                # gather the selected channels, store contiguously
                g = gather_pool.tile([128, 2048], f32, tag=f"g{s}")
                nc.gpsimd.indirect_dma_start(
                    out=g[:], out_offset=None, in_=xg,
                    in_offset=IndirectOffsetOnAxis(ap=idx128_i[:], axis=0))
                nc.sync.dma_start(out=y_d[s * 128:s * 128 + 64, :],
                                  in_=g[0:64, :])
                nc.scalar.dma_start(out=y_d[s * 128 + 64:s * 128 + 128, :],
                                    in_=g[64:128, :])

    nc.compile()
    return nc


def get_nc():
    if "nc" not in _cache:
        _cache["nc"] = _build()
    return _cache["nc"]


def make_in_maps(x: np.ndarray) -> list[dict[str, np.ndarray]]:
    x = np.ascontiguousarray(np.asarray(x, dtype=np.float32))
    assert x.shape == (B, C, H, W)
    return [{"x": x[c * BPC:(c + 1) * BPC].reshape(ROWS, E)}
            for c in range(N_CORES)]


def assemble(results: list[dict[str, np.ndarray]]) -> np.ndarray:
    out = np.empty((B, K, H, W), dtype=np.float32)
    for c in range(N_CORES):
        out[c * BPC:(c + 1) * BPC] = results[c]["y"].reshape(BPC, K, H, W)
    return out


def kernel(x: np.ndarray) -> np.ndarray:
    nc = get_nc()
    res = run_bass_kernel_spmd(nc, make_in_maps(x), list(range(N_CORES)))
    return assemble(res.results)
